# revision 20
# baseline (speedup 1.0000x reference)
"""Dynamic-masked linear (topk_masking) on 8 TRN2 NeuronCores.

Computes reference:
    idx = nonzero(mask)            # exactly K=8192 of 16384
    out = data @ weight[idx].T + bias[idx]     # [8192 tok, 8192 sel]

Strategy (data-parallel over tokens, full selected-weight replicated):
  * Host: nonzero + row-gather of weight/bias (cheap vs 550 GFLOP matmul),
    pack operands into DMA-friendly layouts (partition-major contiguous).
  * Each core m computes out^T[:, m*1024:(m+1)*1024] = W_sel @ X_m^T
    as a PE-stationary-weight matmul: lhsT = W_sel^T tiles [i=128, j=128],
    rhs = X^T tiles [i=128, t=512], accumulating the 32 i-blocks in PSUM
    (fp32 accumulation).
  * Matmul operands are fp16: TRN2 PE streams 16-bit at 1 col/cycle while
    fp32 needs 2 HI/LO passes x 2 cyc/col (4x slower, measured). fp16
    mantissa (10 bits) + fp32 PSUM accumulation keeps scale-relative
    error ~1e-4. An optional 3-pass error-compensated mode (hi*hi +
    hi*lo + lo*hi with fp16 splits) recovers ~fp32 accuracy at 3x PE cost.
  * Bias added during PSUM->SBUF eviction via DVE tensor_scalar_add
    (bias is per-partition in the out^T layout).
  * Host: concat the 8 token-slices of out^T, transpose once.

Per-core: 68.7 GFLOP -> ~0.9 ms PE-bound (fp16); HBM ~104 MiB (hidden).
"""

import contextlib
import sys
import types
from collections import Counter

import numpy as np

import concourse.bacc as bacc
import concourse.bass as bass
import concourse.mybir as mybir
import concourse.tile as tile
from concourse.bass_utils import run_bass_kernel_spmd


def _ensure_axon_hooks():
    """run_bass_kernel_spmd imports antenv.axon_hooks when tracing is
    requested (e.g. BASS_TRACE=1). Some agent images lack that module;
    provide the real ctypes-based hook when possible, else a None hook so
    tracing degrades gracefully instead of crashing the kernel."""
    if "antenv.axon_hooks" in sys.modules:
        return
    try:
        import antenv.axon_hooks  # noqa: F401
        return
    except ImportError:
        pass
    hook = None
    try:
        from trn_agent_boot.trn_boot import _ntff_profile_via_ctypes
        hook = _ntff_profile_via_ctypes("/opt/axon/libaxon_pjrt.so")
    except Exception:
        pass
    mod = types.ModuleType("antenv.axon_hooks")
    mod.get_axon_ntff_profile_hook = lambda: hook
    mod.set_axon_ntff_profile_hook = lambda h: None
    sys.modules["antenv.axon_hooks"] = mod


_ensure_axon_hooks()

N_CORES = 8
P = 128

# Full-problem dims (hardcoded per harness contract)
IN_F = 4096
OUT_F = 16384
N_TOK = 8192
K_SEL = OUT_F // 2
TOK_PER_CORE = N_TOK // N_CORES  # 1024

IB_N = IN_F // P    # 32 contraction blocks
JB_N = K_SEL // P   # 64 output-column panels
TB_SIZE = 512       # moving free dim per matmul (one PSUM bank of fp32)

F32 = mybir.dt.float32
F16 = mybir.dt.float16


def build_program(ib_n=IB_N, jb_n=JB_N, toks=TOK_PER_CORE, tb_size=TB_SIZE,
                  in_dt=F16, split3=False, w_bufs=4):
    """Build the per-core Bass program.

    DRAM parameter layouts (host packs these; `s` below is the split pass
    index — size 2 (hi, lo) when split3 else 1):
      wt  [jb_n, P, s, ib_n, P] : wt[jb, p, s, a, c] = split_s(W_sel)[jb*P+c, a*P+p]
      xt  [P, s, ib_n, toks]    : xt[p, s, a, t]     = split_s(data)[tok0+t, a*P+p]
      bs  [P, jb_n]             : bs[c, jb]          = b_sel[jb*P + c]
      out [jb_n, P, toks]       : out[jb, c, t]      = out^T[jb*P+c, tok0+t]
    """
    s_n = 2 if split3 else 1
    tb_n = toks // tb_size
    assert toks % tb_size == 0

    nc = bacc.Bacc(
        "TRN2", target_bir_lowering=False, debug=False, num_devices=N_CORES
    )
    wt = nc.declare_dram_parameter(
        "wt", [jb_n, P, s_n, ib_n, P], in_dt, isOutput=False)
    xt = nc.declare_dram_parameter(
        "xt", [P, s_n, ib_n, toks], in_dt, isOutput=False)
    bs = nc.declare_dram_parameter("bs", [P, jb_n], F32, isOutput=False)
    out = nc.declare_dram_parameter("out", [jb_n, P, toks], F32, isOutput=True)

    # (stationary split idx, moving split idx) per pass: hi*hi, hi*lo, lo*hi
    passes = [(0, 0), (0, 1), (1, 0)] if split3 else [(0, 0)]

    # x-load chunking: few DMA triggers (issue is ~650 ns each on Sync),
    # ramping sizes so the first matmuls unblock as early as possible.
    if ib_n % 4 == 0 and ib_n >= 8:
        xc_sizes = [1, 1, 2, 4]
        while sum(xc_sizes) < ib_n:
            xc_sizes.append(min(8, ib_n - sum(xc_sizes)))
    else:
        xc_sizes = [1] * ib_n
    xc_start = np.cumsum([0] + xc_sizes)
    ib2chunk = {}
    for c, (st, sz) in enumerate(zip(xc_start, xc_sizes)):
        for k in range(sz):
            ib2chunk[st + k] = (c, k)

    size_counts = Counter(xc_sizes)

    with tile.TileContext(nc) as tc:
        with contextlib.ExitStack() as stk:
            xpools = {
                sz: stk.enter_context(
                    tc.tile_pool(name=f"xpool{sz}", bufs=cnt * s_n))
                for sz, cnt in size_counts.items()
            }
            wpool = stk.enter_context(tc.tile_pool(name="wpool", bufs=w_bufs))
            bpool = stk.enter_context(tc.tile_pool(name="bpool", bufs=1))
            opool = stk.enter_context(tc.tile_pool(name="opool", bufs=4))
            pspool = stk.enter_context(
                tc.tile_pool(name="pspool", bufs=3, space="PSUM"))
            # Prefetch the first weight panels before the (larger) X load so
            # the PE's first matmul isn't gated on a late W DMA.
            w_prefetch = min(2, jb_n)
            w_tiles = []
            for jb in range(w_prefetch):
                w_sb = wpool.tile([P, s_n, ib_n, P], in_dt)
                nc.sync.dma_start(out=w_sb[:], in_=wt[jb])
                w_tiles.append(w_sb)

            # Resident X^T, one tile per (split, chunk-of-i-blocks).
            x_chunks = {}
            for s in range(s_n):
                for c, (st, sz) in enumerate(zip(xc_start, xc_sizes)):
                    x_sb = xpools[sz].tile([P, sz, toks], in_dt)
                    nc.sync.dma_start(
                        out=x_sb[:], in_=xt[:, s, st:st + sz, :])
                    x_chunks[(s, c)] = x_sb

            b_sb = bpool.tile([P, jb_n], F32)
            nc.sync.dma_start(out=b_sb[:], in_=bs[:])

            def x_rhs(s, ib, tb):
                c, k = ib2chunk[ib]
                return x_chunks[(s, c)][
                    :, k, tb * tb_size:(tb + 1) * tb_size]

            for jb in range(jb_n):
                if jb < w_prefetch:
                    w_sb = w_tiles[jb]
                else:
                    w_sb = wpool.tile([P, s_n, ib_n, P], in_dt)
                    nc.sync.dma_start(out=w_sb[:], in_=wt[jb])
                ps = pspool.tile([P, tb_n, tb_size], F32)
                for tb in range(tb_n):
                    n_mm = ib_n * len(passes)
                    k = 0
                    for ib in range(ib_n):
                        for (sw, sx) in passes:
                            nc.tensor.matmul(
                                ps[:, tb, :],
                                w_sb[:, sw, ib, :],
                                x_rhs(sx, ib, tb),
                                start=(k == 0),
                                stop=(k == n_mm - 1),
                            )
                            k += 1
                for tb in range(tb_n):
                    o_sb = opool.tile([P, tb_size], F32)
                    nc.vector.tensor_scalar_add(
                        o_sb[:], ps[:, tb, :], b_sb[:, jb:jb + 1]
                    )
                    nc.sync.dma_start(
                        out=out[jb, :, tb * tb_size:(tb + 1) * tb_size],
                        in_=o_sb[:],
                    )
    nc.compile()
    return nc


_NC_CACHE = {}


def _get_program(in_dt, split3):
    key = (str(in_dt), split3)
    if key not in _NC_CACHE:
        _NC_CACHE[key] = build_program(in_dt=in_dt, split3=split3)
    return _NC_CACHE[key]


def _split_np(a, np_dt, s_n):
    """Return [s_n, ...] stack: hi = cast(a), lo = cast(a - hi)."""
    hi = a.astype(np_dt)
    if s_n == 1:
        return hi[None]
    lo = (a - hi.astype(np.float32)).astype(np_dt)
    return np.stack([hi, lo])


def pack_weight(w_sel, ib_n, jb_n, np_dt=np.float16, s_n=1):
    # w_sel [jb*P+c, a*P+p] -> [s, jb, c, a, p] -> [jb, p, s, a, c]
    ws = _split_np(w_sel, np_dt, s_n).reshape(s_n, jb_n, P, ib_n, P)
    return np.ascontiguousarray(ws.transpose(1, 4, 0, 3, 2))


def pack_x(data_slice, ib_n, toks, np_dt=np.float16, s_n=1):
    # data [t, a*P+p] -> [s, t, a, p] -> [p, s, a, t]
    xs = _split_np(data_slice, np_dt, s_n).reshape(s_n, toks, ib_n, P)
    return np.ascontiguousarray(xs.transpose(3, 0, 2, 1))


def pack_bias(b_sel, jb_n):
    return np.ascontiguousarray(b_sel.reshape(jb_n, P).T.astype(np.float32))


def run(data, weight, bias, mask, trace=False, np_dt=np.float16,
        split3=False):
    """Full pipeline; returns (output, BassKernelResults)."""
    data = np.asarray(data, dtype=np.float32)
    weight = np.asarray(weight, dtype=np.float32)
    bias = np.asarray(bias, dtype=np.float32)
    mask = np.asarray(mask)

    # Mirror jnp.nonzero(mask, size=K)[0]: truncate to the first K hits,
    # pad with index 0 when there are fewer than K.
    idx = np.flatnonzero(mask)
    if idx.size >= K_SEL:
        idx = idx[:K_SEL]
    else:
        idx = np.concatenate(
            [idx, np.zeros(K_SEL - idx.size, dtype=idx.dtype)])
    w_sel = weight[idx]
    b_sel = bias[idx]

    s_n = 2 if split3 else 1
    in_dt = mybir.dt.from_np(np.dtype(np_dt))
    wt_host = pack_weight(w_sel, IB_N, JB_N, np_dt, s_n)
    bs_host = pack_bias(b_sel, JB_N)

    in_maps = []
    for m in range(N_CORES):
        sl = data[m * TOK_PER_CORE:(m + 1) * TOK_PER_CORE]
        in_maps.append({
            "wt": wt_host,
            "xt": pack_x(sl, IB_N, TOK_PER_CORE, np_dt, s_n),
            "bs": bs_host,
        })

    nc = _get_program(in_dt, split3)

    # Host-side spot check rows (one per device) to detect silent output
    # corruption from transient device faults.
    check_rows = [m * TOK_PER_CORE + (m * 131) % TOK_PER_CORE
                  for m in range(N_CORES)]
    exp_rows = data[check_rows] @ w_sel.T + b_sel
    check_tol = 5e-3 * max(np.abs(exp_rows).max(), 1e-30)

    # Transient NRT/device faults (see trn2 pitfalls: "wedged device") can
    # surface as exceptions OR as corrupted output; validate and retry.
    last_err = None
    for attempt in range(3):
        try:
            res = run_bass_kernel_spmd(
                nc, in_maps, list(range(N_CORES)), trace=trace)
            outT = np.concatenate(
                [r["out"].reshape(K_SEL, TOK_PER_CORE) for r in res.results],
                axis=1,
            )
            got_rows = outT[:, check_rows].T
            err = np.abs(got_rows - exp_rows).max()
            if not np.isfinite(err) or err > check_tol:
                raise RuntimeError(
                    f"device output failed validation (err={err:.3e}, "
                    f"tol={check_tol:.3e}); transient fault suspected")
            return np.ascontiguousarray(outT.T), res
        except Exception as e:  # noqa: BLE001
            last_err = e
            import time as _time
            _time.sleep(5)
    raise last_err


def kernel(data, weight, bias, mask):
    out, _ = run(data, weight, bias, mask)
    return out
